# revision 4
# baseline (speedup 1.0000x reference)
"""Barnes-Wall (BW16) lattice quantizer kernel for Trainium2, 8-core data-parallel.

Algorithm (validated bit-exact vs the jax reference):
  x = x_in / a   (correctly-rounded via Dekker-product division: p = x*rh plus
                  exact product error + x*rl correction, rh+rl ~ 1/a in double)
  For each of 32 codebook rows c: v = x - c, g = 2*round(v/2) (RNE, via the
  +1.5*2^24 magic-number trick which rounds v to the nearest even integer),
  eneg = g - v (= X - x, exact), D0 = sum(eneg^2), P2 = sum(g) (exact),
  M = max|eneg|.  Parity (sum f odd) is derived from P2/4 vs its RNE rounding.
  Parity-odd candidates pay a flip penalty: D ~ 4*D0 + odd*(16-16M).
  Winner k = first argmin; its X/eneg/parity are extracted with a one-hot
  masked reduce; the parity flip is applied at the first argmax|eneg|
  coordinate with direction -sign(eneg); y = X' * a.
"""
import sys

sys.path.insert(0, "/opt/trn_rl_repo")
import contextlib

import numpy as np

import concourse.bass as bass
import concourse.bacc as bacc
import concourse.mybir as mybir
import concourse.tile as tile

f32 = np.float32
MAGIC = float(f32(1.5 * 2.0**24))   # round-to-even-integer magic
MAGIC1 = float(f32(1.5 * 2.0**23))  # round-to-integer magic (parity)

dt = mybir.dt
Alu = mybir.AluOpType
Act = mybir.ActivationFunctionType
AX = mybir.AxisListType

N_CORES = 8
R = 4  # row blocks of 128 per iteration


def _bcast(ap, pattern):
    return bass.AP(tensor=ap.tensor, offset=ap.offset, ap=[ap.ap[0]] + pattern)


def _div_consts(a_val):
    """rh + rl ~ 1/a (double-float), rhh + rhl = Veltkamp split of rh."""
    r64 = 1.0 / np.float64(f32(a_val))
    rh = f32(r64)
    rl = f32(r64 - np.float64(rh))
    c_ = f32(rh * f32(4097.0))
    rhh = f32(c_ - f32(c_ - rh))
    rhl = f32(rh - rhh)
    return float(rh), float(rl), float(rhh), float(rhl)


def _build(rows, a_val):
    nc = bacc.Bacc("TRN2", target_bir_lowering=False)
    x_d = nc.dram_tensor("x", [rows, 16], dt.float32, kind="ExternalInput")
    cb_d = nc.dram_tensor("cb", [512], dt.float32, kind="ExternalInput")
    i32_d = nc.dram_tensor("i32", [32], dt.float32, kind="ExternalInput")
    i16_d = nc.dram_tensor("i16", [16], dt.float32, kind="ExternalInput")
    y_d = nc.dram_tensor("y", [rows, 16], dt.float32, kind="ExternalOutput")

    rh, rl, rhh, rhl = _div_consts(a_val)

    n_iters = rows // (128 * R)
    assert n_iters * 128 * R == rows

    with tile.TileContext(nc) as tc:
        with contextlib.ExitStack() as ctx:
            singles = ctx.enter_context(tc.tile_pool(name="singles", bufs=1))
            work = ctx.enter_context(tc.tile_pool(name="work", bufs=2))

            cb_t = singles.tile([128, 512], dt.float32)
            nc.sync.dma_start(out=cb_t, in_=bass.AP(tensor=cb_d, offset=0, ap=[[0, 128], [1, 512]]))
            i32_t = singles.tile([128, 32], dt.float32)
            nc.sync.dma_start(out=i32_t, in_=bass.AP(tensor=i32_d, offset=0, ap=[[0, 128], [1, 32]]))
            i16_t = singles.tile([128, 16], dt.float32)
            nc.sync.dma_start(out=i16_t, in_=bass.AP(tensor=i16_d, offset=0, ap=[[0, 128], [1, 16]]))

            for it in range(n_iters):
                row0 = it * 128 * R
                x_t = work.tile([128, R, 16], dt.float32)
                nc.sync.dma_start(
                    out=x_t,
                    in_=bass.AP(tensor=x_d, offset=row0 * 16, ap=[[16, 128], [128 * 16, R], [1, 16]]),
                )

                # --- xs = x / a, correctly rounded (Dekker product correction) ---
                dk_c = work.tile([128, R, 16], dt.float32)
                nc.vector.tensor_scalar(out=dk_c, in0=x_t, scalar1=4097.0, scalar2=None, op0=Alu.mult)
                dk_u = work.tile([128, R, 16], dt.float32)
                nc.vector.tensor_tensor(out=dk_u, in0=dk_c, in1=x_t, op=Alu.subtract)
                xh = work.tile([128, R, 16], dt.float32)
                nc.vector.tensor_tensor(out=xh, in0=dk_c, in1=dk_u, op=Alu.subtract)
                xl = work.tile([128, R, 16], dt.float32)
                nc.vector.tensor_tensor(out=xl, in0=x_t, in1=xh, op=Alu.subtract)
                dk_p = work.tile([128, R, 16], dt.float32)
                nc.vector.tensor_scalar(out=dk_p, in0=x_t, scalar1=rh, scalar2=None, op0=Alu.mult)
                dk1 = work.tile([128, R, 16], dt.float32)
                nc.vector.scalar_tensor_tensor(out=dk1, in0=xh, scalar=rhh, in1=dk_p, op0=Alu.mult, op1=Alu.subtract)
                dk2 = work.tile([128, R, 16], dt.float32)
                nc.vector.scalar_tensor_tensor(out=dk2, in0=xh, scalar=rhl, in1=dk1, op0=Alu.mult, op1=Alu.add)
                dk3 = work.tile([128, R, 16], dt.float32)
                nc.vector.scalar_tensor_tensor(out=dk3, in0=xl, scalar=rhh, in1=dk2, op0=Alu.mult, op1=Alu.add)
                dk4 = work.tile([128, R, 16], dt.float32)
                nc.vector.scalar_tensor_tensor(out=dk4, in0=xl, scalar=rhl, in1=dk3, op0=Alu.mult, op1=Alu.add)
                dk5 = work.tile([128, R, 16], dt.float32)
                nc.vector.scalar_tensor_tensor(out=dk5, in0=x_t, scalar=rl, in1=dk4, op0=Alu.mult, op1=Alu.add)
                xs = work.tile([128, R, 16], dt.float32)
                nc.vector.tensor_tensor(out=xs, in0=dk_p, in1=dk5, op=Alu.add)

                # --- v = xs - c ---
                v_t = work.tile([128, R, 32, 16], dt.float32)
                xs_b = _bcast(xs, [[16, R], [0, 32], [1, 16]])
                cb_b = _bcast(cb_t, [[0, R], [16, 32], [1, 16]])
                nc.vector.tensor_tensor(out=v_t, in0=xs_b, in1=cb_b, op=Alu.subtract)

                # t = v + MAGIC (ACT); g = t - MAGIC (ACT)
                t_t = work.tile([128, R, 32, 16], dt.float32)
                nc.scalar.activation(out=t_t, in_=v_t, func=Act.Copy, bias=MAGIC, scale=1.0)
                g_t = work.tile([128, R, 32, 16], dt.float32)
                nc.scalar.activation(out=g_t, in_=t_t, func=Act.Copy, bias=-MAGIC, scale=1.0)

                # eneg = g - v   (exact via Sterbenz; = X - x)
                e_t = work.tile([128, R, 32, 16], dt.float32)
                nc.vector.tensor_tensor(out=e_t, in0=g_t, in1=v_t, op=Alu.subtract)

                # sq = eneg^2 (ACT)
                sq_t = work.tile([128, R, 32, 16], dt.float32)
                nc.scalar.activation(out=sq_t, in_=e_t, func=Act.Square, scale=1.0)

                # w = g + c (candidate points X)
                w_t = work.tile([128, R, 32, 16], dt.float32)
                nc.vector.tensor_tensor(out=w_t, in0=g_t, in1=cb_b, op=Alu.add)

                # per-candidate reductions
                D0 = work.tile([128, R, 32], dt.float32)
                nc.vector.tensor_reduce(out=D0, in_=sq_t, axis=AX.X, op=Alu.add)
                P2 = work.tile([128, R, 32], dt.float32)
                nc.vector.tensor_reduce(out=P2, in_=g_t, axis=AX.X, op=Alu.add)
                M = work.tile([128, R, 32], dt.float32)
                nc.vector.tensor_reduce(out=M, in_=e_t, axis=AX.X, op=Alu.max, apply_absolute_value=True)

                # parity: h = P2/4; odd <=> h is an odd multiple of 0.5
                h_t = work.tile([128, R, 32], dt.float32)
                nc.vector.tensor_scalar(out=h_t, in0=P2, scalar1=0.25, scalar2=None, op0=Alu.mult)
                th_t = work.tile([128, R, 32], dt.float32)
                nc.scalar.activation(out=th_t, in_=h_t, func=Act.Copy, bias=MAGIC1, scale=1.0)
                hr_t = work.tile([128, R, 32], dt.float32)
                nc.scalar.activation(out=hr_t, in_=th_t, func=Act.Copy, bias=-MAGIC1, scale=1.0)
                dp_t = work.tile([128, R, 32], dt.float32)
                nc.vector.tensor_tensor(out=dp_t, in0=h_t, in1=hr_t, op=Alu.subtract)
                o2_t = work.tile([128, R, 32], dt.float32)  # 0.25 if odd else 0
                nc.scalar.activation(out=o2_t, in_=dp_t, func=Act.Square, scale=1.0)

                # Dq = 4*D0 + (64 - 64*M) * o2
                W64 = work.tile([128, R, 32], dt.float32)
                nc.vector.tensor_scalar(out=W64, in0=M, scalar1=-64.0, scalar2=64.0, op0=Alu.mult, op1=Alu.add)
                pen = work.tile([128, R, 32], dt.float32)
                nc.vector.tensor_tensor(out=pen, in0=W64, in1=o2_t, op=Alu.mult)
                Dq = work.tile([128, R, 32], dt.float32)
                nc.vector.scalar_tensor_tensor(out=Dq, in0=D0, scalar=4.0, in1=pen, op0=Alu.mult, op1=Alu.add)

                # first argmin -> one-hot
                Dmin = work.tile([128, R], dt.float32)
                nc.vector.tensor_reduce(out=Dmin, in_=Dq, axis=AX.X, op=Alu.min)
                eq = work.tile([128, R, 32], dt.float32)
                nc.vector.tensor_tensor(out=eq, in0=Dq, in1=_bcast(Dmin, [[1, R], [0, 32]]), op=Alu.is_equal)
                m1 = work.tile([128, R, 32], dt.float32)
                nc.vector.tensor_tensor(out=m1, in0=eq, in1=_bcast(i32_t, [[0, R], [1, 32]]), op=Alu.mult)
                km = work.tile([128, R], dt.float32)
                nc.vector.tensor_reduce(out=km, in_=m1, axis=AX.X, op=Alu.min)
                onehot = work.tile([128, R, 32], dt.float32)
                nc.vector.tensor_tensor(
                    out=onehot, in0=_bcast(i32_t, [[0, R], [1, 32]]), in1=_bcast(km, [[1, R], [0, 32]]), op=Alu.is_equal
                )

                # masked selects (transposed write then grouped reduce over k)
                oh_b = _bcast(onehot, [[32, R], [1, 32], [0, 16]])

                wT = work.tile([128, R, 16, 32], dt.float32)
                wT_w = bass.AP(tensor=wT.tensor, offset=wT.offset, ap=[wT.ap[0], [512, R], [1, 32], [32, 16]])
                nc.vector.tensor_tensor(out=wT_w, in0=w_t, in1=oh_b, op=Alu.mult)
                wsel = work.tile([128, R, 16], dt.float32)
                nc.vector.tensor_reduce(out=wsel, in_=wT, axis=AX.X, op=Alu.add)

                eT = work.tile([128, R, 16, 32], dt.float32)
                eT_w = bass.AP(tensor=eT.tensor, offset=eT.offset, ap=[eT.ap[0], [512, R], [1, 32], [32, 16]])
                nc.vector.tensor_tensor(out=eT_w, in0=e_t, in1=oh_b, op=Alu.mult)
                esel = work.tile([128, R, 16], dt.float32)
                nc.vector.tensor_reduce(out=esel, in_=eT, axis=AX.X, op=Alu.add)

                o2m = work.tile([128, R, 32], dt.float32)
                nc.vector.tensor_tensor(out=o2m, in0=o2_t, in1=onehot, op=Alu.mult)
                o2sel = work.tile([128, R], dt.float32)  # 0.25 if odd else 0
                nc.vector.tensor_reduce(out=o2sel, in_=o2m, axis=AX.X, op=Alu.add)

                # parity flip at first argmax|eneg|
                ae = work.tile([128, R, 16], dt.float32)
                nc.scalar.activation(out=ae, in_=esel, func=Act.Abs, scale=1.0)
                M16 = work.tile([128, R], dt.float32)
                nc.vector.tensor_reduce(out=M16, in_=ae, axis=AX.X, op=Alu.max)
                meq = work.tile([128, R, 16], dt.float32)
                nc.vector.tensor_tensor(out=meq, in0=ae, in1=_bcast(M16, [[1, R], [0, 16]]), op=Alu.is_equal)
                m2 = work.tile([128, R, 16], dt.float32)
                nc.vector.tensor_tensor(out=m2, in0=meq, in1=_bcast(i16_t, [[0, R], [1, 16]]), op=Alu.mult)
                jm = work.tile([128, R], dt.float32)
                nc.vector.tensor_reduce(out=jm, in_=m2, axis=AX.X, op=Alu.min)
                mask1 = work.tile([128, R, 16], dt.float32)
                nc.vector.tensor_tensor(
                    out=mask1, in0=_bcast(i16_t, [[0, R], [1, 16]]), in1=_bcast(jm, [[1, R], [0, 16]]), op=Alu.is_equal
                )
                sgn = work.tile([128, R, 16], dt.float32)
                nc.scalar.activation(out=sgn, in_=esel, func=Act.Sign, scale=1.0)
                u1 = work.tile([128, R, 16], dt.float32)
                nc.vector.tensor_tensor(out=u1, in0=mask1, in1=sgn, op=Alu.mult)
                ohalf = work.tile([128, R], dt.float32)  # -2 if odd else 0
                nc.vector.tensor_scalar(out=ohalf, in0=o2sel, scalar1=-8.0, scalar2=None, op0=Alu.mult)
                u2 = work.tile([128, R, 16], dt.float32)
                nc.vector.tensor_tensor(out=u2, in0=u1, in1=_bcast(ohalf, [[1, R], [0, 16]]), op=Alu.mult)
                Xf = work.tile([128, R, 16], dt.float32)
                nc.vector.tensor_tensor(out=Xf, in0=wsel, in1=u2, op=Alu.add)
                y_t = work.tile([128, R, 16], dt.float32)
                nc.vector.tensor_scalar(out=y_t, in0=Xf, scalar1=float(f32(a_val)), scalar2=None, op0=Alu.mult)

                nc.sync.dma_start(
                    out=bass.AP(tensor=y_d, offset=row0 * 16, ap=[[16, 128], [128 * 16, R], [1, 16]]),
                    in_=y_t,
                )
    nc.finalize()
    return nc


_CACHE = {}


def _get_nc(rows, a_val):
    key = (rows, a_val)
    if key not in _CACHE:
        _CACHE[key] = _build(rows, a_val)
    return _CACHE[key]


def kernel(x_in, C_rep, a):
    from concourse.bass_utils import run_bass_kernel_spmd

    x = np.ascontiguousarray(np.asarray(x_in, dtype=np.float32))
    C = np.asarray(C_rep, dtype=np.float32)
    a_val = float(np.asarray(a).reshape(-1)[0])
    B = x.shape[0]
    rows = B // N_CORES
    assert rows * N_CORES == B

    nc = _get_nc(rows, a_val)

    cb_np = C.reshape(-1).astype(np.float32)
    i32_np = (np.arange(32) - 64).astype(np.float32)
    i16_np = (np.arange(16) - 32).astype(np.float32)
    shards = x.reshape(N_CORES, rows, 16)
    in_maps = [
        {"x": shards[i], "cb": cb_np, "i32": i32_np, "i16": i16_np}
        for i in range(N_CORES)
    ]
    res = run_bass_kernel_spmd(nc, in_maps, core_ids=list(range(N_CORES)))
    y = np.concatenate([res.results[i]["y"] for i in range(N_CORES)], axis=0)
    return y.astype(np.float32)


if __name__ == "__main__":
    rng = np.random.default_rng(0)
    x = rng.standard_normal((262144, 16), dtype=np.float32)
    C = rng.integers(0, 5, size=(32, 16)).astype(np.float32)
    a = np.array([0.59460354], dtype=np.float32)
    y = kernel(x, C, a)
    print("ok", y.shape, y.dtype)


# revision 5
# speedup vs baseline: 1.0382x; 1.0382x over previous
"""Barnes-Wall (BW16) lattice quantizer kernel for Trainium2, 8-core data-parallel.

Algorithm (validated bit-exact vs the jax reference):
  x = x_in / a   (correctly-rounded via Dekker-product division: p = x*rh plus
                  exact product error + x*rl correction, rh+rl ~ 1/a in double)
  For each of 32 codebook rows c: v = x - c, g = 2*round(v/2) (RNE, via the
  +1.5*2^24 magic-number trick which rounds v to the nearest even integer),
  eneg = g - v (= X - x, exact), D0 = sum(eneg^2), P2 = sum(g) (exact),
  M = max|eneg|.  Parity (sum f odd) is derived from P2/4 vs its RNE rounding.
  Parity-odd candidates pay a flip penalty: D ~ 4*D0 + odd*(16-16M).
  Winner k = first argmin; its X/eneg/parity are extracted with a one-hot
  masked reduce; the parity flip is applied at the first argmax|eneg|
  coordinate with direction -sign(eneg); y = X' * a.
"""
import sys

sys.path.insert(0, "/opt/trn_rl_repo")
import contextlib

import numpy as np

import concourse.bass as bass
import concourse.bacc as bacc
import concourse.mybir as mybir
import concourse.tile as tile

f32 = np.float32
MAGIC = float(f32(1.5 * 2.0**24))   # round-to-even-integer magic
MAGIC1 = float(f32(1.5 * 2.0**23))  # round-to-integer magic (parity)

dt = mybir.dt
Alu = mybir.AluOpType
Act = mybir.ActivationFunctionType
AX = mybir.AxisListType

N_CORES = 8
R = 4  # row blocks of 128 per iteration


def _bcast(ap, pattern):
    return bass.AP(tensor=ap.tensor, offset=ap.offset, ap=[ap.ap[0]] + pattern)


def _div_consts(a_val):
    """rh + rl ~ 1/a (double-float), rhh + rhl = Veltkamp split of rh."""
    r64 = 1.0 / np.float64(f32(a_val))
    rh = f32(r64)
    rl = f32(r64 - np.float64(rh))
    c_ = f32(rh * f32(4097.0))
    rhh = f32(c_ - f32(c_ - rh))
    rhl = f32(rh - rhh)
    return float(rh), float(rl), float(rhh), float(rhl)


def _build(rows, a_val):
    nc = bacc.Bacc("TRN2", target_bir_lowering=False)
    x_d = nc.dram_tensor("x", [rows, 16], dt.float32, kind="ExternalInput")
    cb_d = nc.dram_tensor("cb", [512], dt.float32, kind="ExternalInput")
    i32_d = nc.dram_tensor("i32", [32], dt.float32, kind="ExternalInput")
    i16_d = nc.dram_tensor("i16", [16], dt.float32, kind="ExternalInput")
    y_d = nc.dram_tensor("y", [rows, 16], dt.float32, kind="ExternalOutput")

    rh, rl, rhh, rhl = _div_consts(a_val)

    n_iters = rows // (128 * R)
    assert n_iters * 128 * R == rows

    with tile.TileContext(nc) as tc:
        with contextlib.ExitStack() as ctx:
            singles = ctx.enter_context(tc.tile_pool(name="singles", bufs=1))
            work = ctx.enter_context(tc.tile_pool(name="work", bufs=2))

            cb_t = singles.tile([128, 512], dt.float32)
            nc.sync.dma_start(out=cb_t, in_=bass.AP(tensor=cb_d, offset=0, ap=[[0, 128], [1, 512]]))
            i32_t = singles.tile([128, 32], dt.float32)
            nc.sync.dma_start(out=i32_t, in_=bass.AP(tensor=i32_d, offset=0, ap=[[0, 128], [1, 32]]))
            i16_t = singles.tile([128, 16], dt.float32)
            nc.sync.dma_start(out=i16_t, in_=bass.AP(tensor=i16_d, offset=0, ap=[[0, 128], [1, 16]]))

            for it in range(n_iters):
                row0 = it * 128 * R
                x_t = work.tile([128, R, 16], dt.float32)
                nc.sync.dma_start(
                    out=x_t,
                    in_=bass.AP(tensor=x_d, offset=row0 * 16, ap=[[16, 128], [128 * 16, R], [1, 16]]),
                )

                # --- xs = x / a, correctly rounded (Dekker product correction) ---
                dk_c = work.tile([128, R, 16], dt.float32)
                nc.vector.tensor_scalar(out=dk_c, in0=x_t, scalar1=4097.0, scalar2=None, op0=Alu.mult)
                dk_u = work.tile([128, R, 16], dt.float32)
                nc.vector.tensor_tensor(out=dk_u, in0=dk_c, in1=x_t, op=Alu.subtract)
                xh = work.tile([128, R, 16], dt.float32)
                nc.vector.tensor_tensor(out=xh, in0=dk_c, in1=dk_u, op=Alu.subtract)
                xl = work.tile([128, R, 16], dt.float32)
                nc.vector.tensor_tensor(out=xl, in0=x_t, in1=xh, op=Alu.subtract)
                dk_p = work.tile([128, R, 16], dt.float32)
                nc.vector.tensor_scalar(out=dk_p, in0=x_t, scalar1=rh, scalar2=None, op0=Alu.mult)
                dk1 = work.tile([128, R, 16], dt.float32)
                nc.vector.scalar_tensor_tensor(out=dk1, in0=xh, scalar=rhh, in1=dk_p, op0=Alu.mult, op1=Alu.subtract)
                dk2 = work.tile([128, R, 16], dt.float32)
                nc.vector.scalar_tensor_tensor(out=dk2, in0=xh, scalar=rhl, in1=dk1, op0=Alu.mult, op1=Alu.add)
                dk3 = work.tile([128, R, 16], dt.float32)
                nc.vector.scalar_tensor_tensor(out=dk3, in0=xl, scalar=rhh, in1=dk2, op0=Alu.mult, op1=Alu.add)
                dk4 = work.tile([128, R, 16], dt.float32)
                nc.vector.scalar_tensor_tensor(out=dk4, in0=xl, scalar=rhl, in1=dk3, op0=Alu.mult, op1=Alu.add)
                dk5 = work.tile([128, R, 16], dt.float32)
                nc.vector.scalar_tensor_tensor(out=dk5, in0=x_t, scalar=rl, in1=dk4, op0=Alu.mult, op1=Alu.add)
                xs = work.tile([128, R, 16], dt.float32)
                nc.vector.tensor_tensor(out=xs, in0=dk_p, in1=dk5, op=Alu.add)

                # --- v = xs - c ---
                v_t = work.tile([128, R, 32, 16], dt.float32)
                xs_b = _bcast(xs, [[16, R], [0, 32], [1, 16]])
                cb_b = _bcast(cb_t, [[0, R], [16, 32], [1, 16]])
                nc.vector.tensor_tensor(out=v_t, in0=xs_b, in1=cb_b, op=Alu.subtract)

                # t = v + MAGIC (ACT); g = t - MAGIC (ACT)
                t_t = work.tile([128, R, 32, 16], dt.float32)
                nc.scalar.activation(out=t_t, in_=v_t, func=Act.Copy, bias=MAGIC, scale=1.0)
                g_t = work.tile([128, R, 32, 16], dt.float32)
                nc.scalar.activation(out=g_t, in_=t_t, func=Act.Copy, bias=-MAGIC, scale=1.0)

                # eneg = g - v   (exact via Sterbenz; = X - x)
                e_t = work.tile([128, R, 32, 16], dt.float32)
                nc.vector.tensor_tensor(out=e_t, in0=g_t, in1=v_t, op=Alu.subtract)

                # sq = eneg^2 (ACT)
                sq_t = work.tile([128, R, 32, 16], dt.float32)
                nc.scalar.activation(out=sq_t, in_=e_t, func=Act.Square, scale=1.0)

                # w = g + c (candidate points X)
                w_t = work.tile([128, R, 32, 16], dt.float32)
                nc.vector.tensor_tensor(out=w_t, in0=g_t, in1=cb_b, op=Alu.add)

                # per-candidate reductions
                D0 = work.tile([128, R, 32], dt.float32)
                nc.vector.tensor_reduce(out=D0, in_=sq_t, axis=AX.X, op=Alu.add)
                P2 = work.tile([128, R, 32], dt.float32)
                nc.vector.tensor_reduce(out=P2, in_=g_t, axis=AX.X, op=Alu.add)
                M = work.tile([128, R, 32], dt.float32)
                nc.vector.tensor_reduce(out=M, in_=e_t, axis=AX.X, op=Alu.max, apply_absolute_value=True)

                # parity: h = P2/4; odd <=> h is an odd multiple of 0.5
                h_t = work.tile([128, R, 32], dt.float32)
                nc.vector.tensor_scalar(out=h_t, in0=P2, scalar1=0.25, scalar2=None, op0=Alu.mult)
                th_t = work.tile([128, R, 32], dt.float32)
                nc.scalar.activation(out=th_t, in_=h_t, func=Act.Copy, bias=MAGIC1, scale=1.0)
                hr_t = work.tile([128, R, 32], dt.float32)
                nc.scalar.activation(out=hr_t, in_=th_t, func=Act.Copy, bias=-MAGIC1, scale=1.0)
                dp_t = work.tile([128, R, 32], dt.float32)
                nc.vector.tensor_tensor(out=dp_t, in0=h_t, in1=hr_t, op=Alu.subtract)
                o2_t = work.tile([128, R, 32], dt.float32)  # 0.25 if odd else 0
                nc.scalar.activation(out=o2_t, in_=dp_t, func=Act.Square, scale=1.0)

                # Dq = 4*D0 + (64 - 64*M) * o2
                W64 = work.tile([128, R, 32], dt.float32)
                nc.vector.tensor_scalar(out=W64, in0=M, scalar1=-64.0, scalar2=64.0, op0=Alu.mult, op1=Alu.add)
                pen = work.tile([128, R, 32], dt.float32)
                nc.vector.tensor_tensor(out=pen, in0=W64, in1=o2_t, op=Alu.mult)
                Dq = work.tile([128, R, 32], dt.float32)
                nc.vector.scalar_tensor_tensor(out=Dq, in0=D0, scalar=4.0, in1=pen, op0=Alu.mult, op1=Alu.add)

                # first argmin -> one-hot
                Dmin = work.tile([128, R], dt.float32)
                nc.vector.tensor_reduce(out=Dmin, in_=Dq, axis=AX.X, op=Alu.min)
                eq = work.tile([128, R, 32], dt.float32)
                nc.vector.tensor_tensor(out=eq, in0=Dq, in1=_bcast(Dmin, [[1, R], [0, 32]]), op=Alu.is_equal)
                m1 = work.tile([128, R, 32], dt.float32)
                nc.vector.tensor_tensor(out=m1, in0=eq, in1=_bcast(i32_t, [[0, R], [1, 32]]), op=Alu.mult)
                km = work.tile([128, R], dt.float32)
                nc.vector.tensor_reduce(out=km, in_=m1, axis=AX.X, op=Alu.min)
                onehot = work.tile([128, R, 32], dt.float32)
                nc.vector.tensor_tensor(
                    out=onehot, in0=_bcast(i32_t, [[0, R], [1, 32]]), in1=_bcast(km, [[1, R], [0, 32]]), op=Alu.is_equal
                )

                # masked selects (transposed write then grouped reduce over k)
                oh_b = _bcast(onehot, [[32, R], [1, 32], [0, 16]])

                wT = work.tile([128, R, 16, 32], dt.float32)
                wT_w = bass.AP(tensor=wT.tensor, offset=wT.offset, ap=[wT.ap[0], [512, R], [1, 32], [32, 16]])
                nc.gpsimd.tensor_tensor(out=wT_w, in0=w_t, in1=oh_b, op=Alu.mult)
                wsel = work.tile([128, R, 16], dt.float32)
                nc.vector.tensor_reduce(out=wsel, in_=wT, axis=AX.X, op=Alu.add)

                eT = work.tile([128, R, 16, 32], dt.float32)
                eT_w = bass.AP(tensor=eT.tensor, offset=eT.offset, ap=[eT.ap[0], [512, R], [1, 32], [32, 16]])
                nc.gpsimd.tensor_tensor(out=eT_w, in0=e_t, in1=oh_b, op=Alu.mult)
                esel = work.tile([128, R, 16], dt.float32)
                nc.vector.tensor_reduce(out=esel, in_=eT, axis=AX.X, op=Alu.add)

                o2m = work.tile([128, R, 32], dt.float32)
                nc.gpsimd.tensor_tensor(out=o2m, in0=o2_t, in1=onehot, op=Alu.mult)
                o2sel = work.tile([128, R], dt.float32)  # 0.25 if odd else 0
                nc.vector.tensor_reduce(out=o2sel, in_=o2m, axis=AX.X, op=Alu.add)

                # parity flip at first argmax|eneg|
                ae = work.tile([128, R, 16], dt.float32)
                nc.scalar.activation(out=ae, in_=esel, func=Act.Abs, scale=1.0)
                M16 = work.tile([128, R], dt.float32)
                nc.vector.tensor_reduce(out=M16, in_=ae, axis=AX.X, op=Alu.max)
                meq = work.tile([128, R, 16], dt.float32)
                nc.vector.tensor_tensor(out=meq, in0=ae, in1=_bcast(M16, [[1, R], [0, 16]]), op=Alu.is_equal)
                m2 = work.tile([128, R, 16], dt.float32)
                nc.vector.tensor_tensor(out=m2, in0=meq, in1=_bcast(i16_t, [[0, R], [1, 16]]), op=Alu.mult)
                jm = work.tile([128, R], dt.float32)
                nc.vector.tensor_reduce(out=jm, in_=m2, axis=AX.X, op=Alu.min)
                mask1 = work.tile([128, R, 16], dt.float32)
                nc.vector.tensor_tensor(
                    out=mask1, in0=_bcast(i16_t, [[0, R], [1, 16]]), in1=_bcast(jm, [[1, R], [0, 16]]), op=Alu.is_equal
                )
                sgn = work.tile([128, R, 16], dt.float32)
                nc.scalar.activation(out=sgn, in_=esel, func=Act.Sign, scale=1.0)
                u1 = work.tile([128, R, 16], dt.float32)
                nc.vector.tensor_tensor(out=u1, in0=mask1, in1=sgn, op=Alu.mult)
                ohalf = work.tile([128, R], dt.float32)  # -2 if odd else 0
                nc.vector.tensor_scalar(out=ohalf, in0=o2sel, scalar1=-8.0, scalar2=None, op0=Alu.mult)
                u2 = work.tile([128, R, 16], dt.float32)
                nc.vector.tensor_tensor(out=u2, in0=u1, in1=_bcast(ohalf, [[1, R], [0, 16]]), op=Alu.mult)
                Xf = work.tile([128, R, 16], dt.float32)
                nc.vector.tensor_tensor(out=Xf, in0=wsel, in1=u2, op=Alu.add)
                y_t = work.tile([128, R, 16], dt.float32)
                nc.vector.tensor_scalar(out=y_t, in0=Xf, scalar1=float(f32(a_val)), scalar2=None, op0=Alu.mult)

                nc.sync.dma_start(
                    out=bass.AP(tensor=y_d, offset=row0 * 16, ap=[[16, 128], [128 * 16, R], [1, 16]]),
                    in_=y_t,
                )
    nc.finalize()
    return nc


_CACHE = {}


def _get_nc(rows, a_val):
    key = (rows, a_val)
    if key not in _CACHE:
        _CACHE[key] = _build(rows, a_val)
    return _CACHE[key]


def kernel(x_in, C_rep, a):
    from concourse.bass_utils import run_bass_kernel_spmd

    x = np.ascontiguousarray(np.asarray(x_in, dtype=np.float32))
    C = np.asarray(C_rep, dtype=np.float32)
    a_val = float(np.asarray(a).reshape(-1)[0])
    B = x.shape[0]
    rows = B // N_CORES
    assert rows * N_CORES == B

    nc = _get_nc(rows, a_val)

    cb_np = C.reshape(-1).astype(np.float32)
    i32_np = (np.arange(32) - 64).astype(np.float32)
    i16_np = (np.arange(16) - 32).astype(np.float32)
    shards = x.reshape(N_CORES, rows, 16)
    in_maps = [
        {"x": shards[i], "cb": cb_np, "i32": i32_np, "i16": i16_np}
        for i in range(N_CORES)
    ]
    res = run_bass_kernel_spmd(nc, in_maps, core_ids=list(range(N_CORES)))
    y = np.concatenate([res.results[i]["y"] for i in range(N_CORES)], axis=0)
    return y.astype(np.float32)


if __name__ == "__main__":
    rng = np.random.default_rng(0)
    x = rng.standard_normal((262144, 16), dtype=np.float32)
    C = rng.integers(0, 5, size=(32, 16)).astype(np.float32)
    a = np.array([0.59460354], dtype=np.float32)
    y = kernel(x, C, a)
    print("ok", y.shape, y.dtype)


# revision 9
# speedup vs baseline: 1.1077x; 1.0669x over previous
"""Barnes-Wall (BW16) lattice quantizer kernel for Trainium2, 8-core data-parallel.

Algorithm (validated bit-exact vs the jax reference):
  x = x_in / a   (correctly-rounded via Dekker-product division: p = x*rh plus
                  exact product error + x*rl correction, rh+rl ~ 1/a in double)
  For each of 32 codebook rows c: v = x - c, g = 2*round(v/2) (RNE, via the
  +1.5*2^24 magic-number trick which rounds v to the nearest even integer),
  eneg = g - v (= X - x, exact), D0 = sum(eneg^2), P2 = sum(g) (exact),
  M = max|eneg|.  Parity (sum f odd) is derived from P2/4 vs its RNE rounding.
  Parity-odd candidates pay a flip penalty: D ~ 4*D0 + odd*(16-16M).
  Winner k = first argmin; its X/eneg/parity are extracted with a one-hot
  masked reduce; the parity flip is applied at the first argmax|eneg|
  coordinate with direction -sign(eneg); y = X' * a.
"""
import sys

sys.path.insert(0, "/opt/trn_rl_repo")
import contextlib

import numpy as np

import concourse.bass as bass
import concourse.bacc as bacc
import concourse.mybir as mybir
import concourse.tile as tile

f32 = np.float32
MAGIC = float(f32(1.5 * 2.0**24))   # round-to-even-integer magic
MAGIC1 = float(f32(1.5 * 2.0**23))  # round-to-integer magic (parity)

dt = mybir.dt
Alu = mybir.AluOpType
Act = mybir.ActivationFunctionType
AX = mybir.AxisListType

N_CORES = 8
R = 4  # row blocks of 128 per iteration


def _bcast(ap, pattern):
    return bass.AP(tensor=ap.tensor, offset=ap.offset, ap=[ap.ap[0]] + pattern)


def _div_consts(a_val):
    """rh + rl ~ 1/a (double-float), rhh + rhl = Veltkamp split of rh."""
    r64 = 1.0 / np.float64(f32(a_val))
    rh = f32(r64)
    rl = f32(r64 - np.float64(rh))
    c_ = f32(rh * f32(4097.0))
    rhh = f32(c_ - f32(c_ - rh))
    rhl = f32(rh - rhh)
    return float(rh), float(rl), float(rhh), float(rhl)


def _build(rows, a_val):
    nc = bacc.Bacc("TRN2", target_bir_lowering=False)
    x_d = nc.dram_tensor("x", [rows, 16], dt.float32, kind="ExternalInput")
    cb_d = nc.dram_tensor("cb", [512], dt.float32, kind="ExternalInput")
    i32_d = nc.dram_tensor("i32", [32], dt.float32, kind="ExternalInput")
    i16_d = nc.dram_tensor("i16", [16], dt.float32, kind="ExternalInput")
    y_d = nc.dram_tensor("y", [rows, 16], dt.float32, kind="ExternalOutput")

    rh, rl, rhh, rhl = _div_consts(a_val)

    n_iters = rows // (128 * R)
    assert n_iters * 128 * R == rows

    with tile.TileContext(nc) as tc:
        with contextlib.ExitStack() as ctx:
            singles = ctx.enter_context(tc.tile_pool(name="singles", bufs=1))

            cb_t = singles.tile([128, 512], dt.float32)
            nc.sync.dma_start(out=cb_t, in_=bass.AP(tensor=cb_d, offset=0, ap=[[0, 128], [1, 512]]))
            i32_t = singles.tile([128, 32], dt.float32)
            nc.sync.dma_start(out=i32_t, in_=bass.AP(tensor=i32_d, offset=0, ap=[[0, 128], [1, 32]]))
            i16_t = singles.tile([128, 16], dt.float32)
            nc.sync.dma_start(out=i16_t, in_=bass.AP(tensor=i16_d, offset=0, ap=[[0, 128], [1, 16]]))

            # prologue: xs_all = x / a for the whole shard (Dekker, correctly rounded)
            nrb = rows // 128
            xs_all = singles.tile([128, nrb, 16], dt.float32)
            dkpool_cm = tc.tile_pool(name="dk", bufs=1)
            dkpool = dkpool_cm.__enter__()
            x_all = dkpool.tile([128, nrb, 16], dt.float32)
            nc.sync.dma_start(out=x_all, in_=bass.AP(tensor=x_d, offset=0, ap=[[16, 128], [128 * 16, nrb], [1, 16]]))
            dkA = dkpool.tile([128, nrb, 16], dt.float32)
            dkB = dkpool.tile([128, nrb, 16], dt.float32)
            dkC2 = dkpool.tile([128, nrb, 16], dt.float32)
            dkD = dkpool.tile([128, nrb, 16], dt.float32)
            nc.vector.tensor_scalar(out=dkA, in0=x_all, scalar1=4097.0, scalar2=None, op0=Alu.mult)      # c
            nc.vector.tensor_tensor(out=dkB, in0=dkA, in1=x_all, op=Alu.subtract)                        # u = c - x
            nc.vector.tensor_tensor(out=dkA, in0=dkA, in1=dkB, op=Alu.subtract)                          # xh = c - u
            nc.vector.tensor_tensor(out=dkB, in0=x_all, in1=dkA, op=Alu.subtract)                        # xl
            nc.vector.tensor_scalar(out=dkC2, in0=x_all, scalar1=rh, scalar2=None, op0=Alu.mult)         # p
            nc.vector.scalar_tensor_tensor(out=dkD, in0=dkA, scalar=rhh, in1=dkC2, op0=Alu.mult, op1=Alu.subtract)
            nc.vector.scalar_tensor_tensor(out=dkD, in0=dkA, scalar=rhl, in1=dkD, op0=Alu.mult, op1=Alu.add)
            nc.vector.scalar_tensor_tensor(out=dkD, in0=dkB, scalar=rhh, in1=dkD, op0=Alu.mult, op1=Alu.add)
            nc.vector.scalar_tensor_tensor(out=dkD, in0=dkB, scalar=rhl, in1=dkD, op0=Alu.mult, op1=Alu.add)
            nc.vector.scalar_tensor_tensor(out=dkD, in0=x_all, scalar=rl, in1=dkD, op0=Alu.mult, op1=Alu.add)
            nc.vector.tensor_tensor(out=xs_all, in0=dkC2, in1=dkD, op=Alu.add)
            dkpool_cm.__exit__(None, None, None)
            work = ctx.enter_context(tc.tile_pool(name="work", bufs=3))

            for it in range(n_iters):
                row0 = it * 128 * R
                # --- v = xs - c ---
                v_t = work.tile([128, R, 32, 16], dt.float32)
                xs_sl = xs_all[:, it * R:(it + 1) * R, :]
                xs_b = bass.AP(tensor=xs_sl.tensor, offset=xs_sl.offset, ap=[xs_sl.ap[0], [16, R], [0, 32], [1, 16]])
                cb_b = _bcast(cb_t, [[0, R], [16, 32], [1, 16]])
                nc.vector.tensor_tensor(out=v_t, in0=xs_b, in1=cb_b, op=Alu.subtract)

                # t = v + MAGIC (ACT); g = t - MAGIC (ACT)
                t_t = work.tile([128, R, 32, 16], dt.float32)
                nc.scalar.activation(out=t_t, in_=v_t, func=Act.Copy, bias=MAGIC, scale=1.0)
                g_t = work.tile([128, R, 32, 16], dt.float32)
                nc.scalar.activation(out=g_t, in_=t_t, func=Act.Copy, bias=-MAGIC, scale=1.0)

                # eneg = g - v   (exact via Sterbenz; = X - x); overwrites v in place
                e_t = v_t
                nc.vector.tensor_tensor(out=e_t, in0=g_t, in1=v_t, op=Alu.subtract)

                # sq = eneg^2 (ACT); overwrites t in place
                sq_t = t_t
                nc.scalar.activation(out=sq_t, in_=e_t, func=Act.Square, scale=1.0)

                # per-candidate reductions
                D0 = work.tile([128, R, 32], dt.float32)
                nc.vector.tensor_reduce(out=D0, in_=sq_t, axis=AX.X, op=Alu.add)
                P2 = work.tile([128, R, 32], dt.float32)
                nc.vector.tensor_reduce(out=P2, in_=g_t, axis=AX.X, op=Alu.add)
                M = work.tile([128, R, 32], dt.float32)
                nc.vector.tensor_reduce(out=M, in_=e_t, axis=AX.X, op=Alu.max, apply_absolute_value=True)

                # w = g + c (candidate points X); overwrites g in place
                w_t = g_t
                nc.vector.tensor_tensor(out=w_t, in0=g_t, in1=cb_b, op=Alu.add)


                # parity: h = P2/4; odd <=> h is an odd multiple of 0.5
                h_t = work.tile([128, R, 32], dt.float32)
                nc.vector.tensor_scalar(out=h_t, in0=P2, scalar1=0.25, scalar2=None, op0=Alu.mult)
                th_t = work.tile([128, R, 32], dt.float32)
                nc.scalar.activation(out=th_t, in_=h_t, func=Act.Copy, bias=MAGIC1, scale=1.0)
                hr_t = work.tile([128, R, 32], dt.float32)
                nc.scalar.activation(out=hr_t, in_=th_t, func=Act.Copy, bias=-MAGIC1, scale=1.0)
                dp_t = work.tile([128, R, 32], dt.float32)
                nc.vector.tensor_tensor(out=dp_t, in0=h_t, in1=hr_t, op=Alu.subtract)
                o2_t = work.tile([128, R, 32], dt.float32)  # 0.25 if odd else 0
                nc.scalar.activation(out=o2_t, in_=dp_t, func=Act.Square, scale=1.0)

                # Dq = 4*D0 + (64 - 64*M) * o2
                W64 = work.tile([128, R, 32], dt.float32)
                nc.vector.tensor_scalar(out=W64, in0=M, scalar1=-64.0, scalar2=64.0, op0=Alu.mult, op1=Alu.add)
                pen = work.tile([128, R, 32], dt.float32)
                nc.vector.tensor_tensor(out=pen, in0=W64, in1=o2_t, op=Alu.mult)
                Dq = work.tile([128, R, 32], dt.float32)
                nc.vector.scalar_tensor_tensor(out=Dq, in0=D0, scalar=4.0, in1=pen, op0=Alu.mult, op1=Alu.add)

                # first argmin -> one-hot
                Dmin = work.tile([128, R], dt.float32)
                nc.vector.tensor_reduce(out=Dmin, in_=Dq, axis=AX.X, op=Alu.min)
                eq = work.tile([128, R, 32], dt.float32)
                nc.vector.tensor_tensor(out=eq, in0=Dq, in1=_bcast(Dmin, [[1, R], [0, 32]]), op=Alu.is_equal)
                m1 = work.tile([128, R, 32], dt.float32)
                nc.vector.tensor_tensor(out=m1, in0=eq, in1=_bcast(i32_t, [[0, R], [1, 32]]), op=Alu.mult)
                km = work.tile([128, R], dt.float32)
                nc.vector.tensor_reduce(out=km, in_=m1, axis=AX.X, op=Alu.min)
                onehot = work.tile([128, R, 32], dt.float32)
                nc.vector.tensor_tensor(
                    out=onehot, in0=_bcast(i32_t, [[0, R], [1, 32]]), in1=_bcast(km, [[1, R], [0, 32]]), op=Alu.is_equal
                )

                # masked selects (transposed write then grouped reduce over k)
                oh_b = _bcast(onehot, [[32, R], [1, 32], [0, 16]])

                wT = work.tile([128, R, 16, 32], dt.float32)
                wT_w = bass.AP(tensor=wT.tensor, offset=wT.offset, ap=[wT.ap[0], [512, R], [1, 32], [32, 16]])
                nc.gpsimd.tensor_tensor(out=wT_w, in0=w_t, in1=oh_b, op=Alu.mult)
                wsel = work.tile([128, R, 16], dt.float32)
                nc.vector.tensor_reduce(out=wsel, in_=wT, axis=AX.X, op=Alu.add)

                eT = work.tile([128, R, 16, 32], dt.float32)
                eT_w = bass.AP(tensor=eT.tensor, offset=eT.offset, ap=[eT.ap[0], [512, R], [1, 32], [32, 16]])
                nc.gpsimd.tensor_tensor(out=eT_w, in0=e_t, in1=oh_b, op=Alu.mult)
                esel = work.tile([128, R, 16], dt.float32)
                nc.vector.tensor_reduce(out=esel, in_=eT, axis=AX.X, op=Alu.add)

                o2m = work.tile([128, R, 32], dt.float32)
                nc.gpsimd.tensor_tensor(out=o2m, in0=o2_t, in1=onehot, op=Alu.mult)
                o2sel = work.tile([128, R], dt.float32)  # 0.25 if odd else 0
                nc.vector.tensor_reduce(out=o2sel, in_=o2m, axis=AX.X, op=Alu.add)

                # parity flip at first argmax|eneg|
                ae = work.tile([128, R, 16], dt.float32)
                nc.scalar.activation(out=ae, in_=esel, func=Act.Abs, scale=1.0)
                M16 = work.tile([128, R], dt.float32)
                nc.vector.tensor_reduce(out=M16, in_=ae, axis=AX.X, op=Alu.max)
                meq = work.tile([128, R, 16], dt.float32)
                nc.vector.tensor_tensor(out=meq, in0=ae, in1=_bcast(M16, [[1, R], [0, 16]]), op=Alu.is_equal)
                m2 = work.tile([128, R, 16], dt.float32)
                nc.vector.tensor_tensor(out=m2, in0=meq, in1=_bcast(i16_t, [[0, R], [1, 16]]), op=Alu.mult)
                jm = work.tile([128, R], dt.float32)
                nc.vector.tensor_reduce(out=jm, in_=m2, axis=AX.X, op=Alu.min)
                mask1 = work.tile([128, R, 16], dt.float32)
                nc.vector.tensor_tensor(
                    out=mask1, in0=_bcast(i16_t, [[0, R], [1, 16]]), in1=_bcast(jm, [[1, R], [0, 16]]), op=Alu.is_equal
                )
                sgn = work.tile([128, R, 16], dt.float32)
                nc.scalar.activation(out=sgn, in_=esel, func=Act.Sign, scale=1.0)
                u1 = work.tile([128, R, 16], dt.float32)
                nc.vector.tensor_tensor(out=u1, in0=mask1, in1=sgn, op=Alu.mult)
                ohalf = work.tile([128, R], dt.float32)  # -2 if odd else 0
                nc.vector.tensor_scalar(out=ohalf, in0=o2sel, scalar1=-8.0, scalar2=None, op0=Alu.mult)
                u2 = work.tile([128, R, 16], dt.float32)
                nc.vector.tensor_tensor(out=u2, in0=u1, in1=_bcast(ohalf, [[1, R], [0, 16]]), op=Alu.mult)
                Xf = work.tile([128, R, 16], dt.float32)
                nc.vector.tensor_tensor(out=Xf, in0=wsel, in1=u2, op=Alu.add)
                y_t = work.tile([128, R, 16], dt.float32)
                nc.vector.tensor_scalar(out=y_t, in0=Xf, scalar1=float(f32(a_val)), scalar2=None, op0=Alu.mult)

                nc.sync.dma_start(
                    out=bass.AP(tensor=y_d, offset=row0 * 16, ap=[[16, 128], [128 * 16, R], [1, 16]]),
                    in_=y_t,
                )
    nc.finalize()
    return nc


_CACHE = {}


def _get_nc(rows, a_val):
    key = (rows, a_val)
    if key not in _CACHE:
        _CACHE[key] = _build(rows, a_val)
    return _CACHE[key]


def kernel(x_in, C_rep, a):
    from concourse.bass_utils import run_bass_kernel_spmd

    x = np.ascontiguousarray(np.asarray(x_in, dtype=np.float32))
    C = np.asarray(C_rep, dtype=np.float32)
    a_val = float(np.asarray(a).reshape(-1)[0])
    B = x.shape[0]
    rows = B // N_CORES
    assert rows * N_CORES == B

    nc = _get_nc(rows, a_val)

    cb_np = C.reshape(-1).astype(np.float32)
    i32_np = (np.arange(32) - 64).astype(np.float32)
    i16_np = (np.arange(16) - 32).astype(np.float32)
    shards = x.reshape(N_CORES, rows, 16)
    in_maps = [
        {"x": shards[i], "cb": cb_np, "i32": i32_np, "i16": i16_np}
        for i in range(N_CORES)
    ]
    res = run_bass_kernel_spmd(nc, in_maps, core_ids=list(range(N_CORES)))
    y = np.concatenate([res.results[i]["y"] for i in range(N_CORES)], axis=0)
    return y.astype(np.float32)


if __name__ == "__main__":
    rng = np.random.default_rng(0)
    x = rng.standard_normal((262144, 16), dtype=np.float32)
    C = rng.integers(0, 5, size=(32, 16)).astype(np.float32)
    a = np.array([0.59460354], dtype=np.float32)
    y = kernel(x, C, a)
    print("ok", y.shape, y.dtype)


# revision 10
# speedup vs baseline: 1.1105x; 1.0025x over previous
"""Barnes-Wall (BW16) lattice quantizer kernel for Trainium2, 8-core data-parallel.

Algorithm (validated bit-exact vs the jax reference):
  x = x_in / a   (correctly-rounded via Dekker-product division: p = x*rh plus
                  exact product error + x*rl correction, rh+rl ~ 1/a in double)
  For each of 32 codebook rows c: v = x - c, g = 2*round(v/2) (RNE, via the
  +1.5*2^24 magic-number trick which rounds v to the nearest even integer),
  eneg = g - v (= X - x, exact), D0 = sum(eneg^2), P2 = sum(g) (exact),
  M = max|eneg|.  Parity (sum f odd) is derived from P2/4 vs its RNE rounding.
  Parity-odd candidates pay a flip penalty: D ~ 4*D0 + odd*(16-16M).
  Winner k = first argmin; its X/eneg/parity are extracted with a one-hot
  masked reduce; the parity flip is applied at the first argmax|eneg|
  coordinate with direction -sign(eneg); y = X' * a.
"""
import sys

sys.path.insert(0, "/opt/trn_rl_repo")
import contextlib

import numpy as np

import concourse.bass as bass
import concourse.bacc as bacc
import concourse.mybir as mybir
import concourse.tile as tile

f32 = np.float32
MAGIC = float(f32(1.5 * 2.0**24))   # round-to-even-integer magic
MAGIC1 = float(f32(1.5 * 2.0**23))  # round-to-integer magic (parity)

dt = mybir.dt
Alu = mybir.AluOpType
Act = mybir.ActivationFunctionType
AX = mybir.AxisListType

N_CORES = 8
R = 4  # row blocks of 128 per iteration


def _bcast(ap, pattern):
    return bass.AP(tensor=ap.tensor, offset=ap.offset, ap=[ap.ap[0]] + pattern)


def _div_consts(a_val):
    """rh + rl ~ 1/a (double-float), rhh + rhl = Veltkamp split of rh."""
    r64 = 1.0 / np.float64(f32(a_val))
    rh = f32(r64)
    rl = f32(r64 - np.float64(rh))
    c_ = f32(rh * f32(4097.0))
    rhh = f32(c_ - f32(c_ - rh))
    rhl = f32(rh - rhh)
    return float(rh), float(rl), float(rhh), float(rhl)


def _build(rows, a_val):
    nc = bacc.Bacc("TRN2", target_bir_lowering=False)
    x_d = nc.dram_tensor("x", [rows, 16], dt.float32, kind="ExternalInput")
    cb_d = nc.dram_tensor("cb", [512], dt.float32, kind="ExternalInput")
    i32_d = nc.dram_tensor("i32", [32], dt.float32, kind="ExternalInput")
    i16_d = nc.dram_tensor("i16", [16], dt.float32, kind="ExternalInput")
    y_d = nc.dram_tensor("y", [rows, 16], dt.float32, kind="ExternalOutput")

    rh, rl, rhh, rhl = _div_consts(a_val)

    n_iters = rows // (128 * R)
    assert n_iters * 128 * R == rows

    with tile.TileContext(nc) as tc:
        with contextlib.ExitStack() as ctx:
            singles = ctx.enter_context(tc.tile_pool(name="singles", bufs=1))

            cb_t = singles.tile([128, 512], dt.float32)
            nc.sync.dma_start(out=cb_t, in_=bass.AP(tensor=cb_d, offset=0, ap=[[0, 128], [1, 512]]))
            i32_t = singles.tile([128, 32], dt.float32)
            nc.sync.dma_start(out=i32_t, in_=bass.AP(tensor=i32_d, offset=0, ap=[[0, 128], [1, 32]]))
            i16_t = singles.tile([128, 16], dt.float32)
            nc.sync.dma_start(out=i16_t, in_=bass.AP(tensor=i16_d, offset=0, ap=[[0, 128], [1, 16]]))

            # prologue: xs_all = x / a for the whole shard (Dekker, correctly rounded)
            nrb = rows // 128
            xs_all = singles.tile([128, nrb, 16], dt.float32)
            dkpool_cm = tc.tile_pool(name="dk", bufs=1)
            dkpool = dkpool_cm.__enter__()
            x_all = dkpool.tile([128, nrb, 16], dt.float32)
            nc.sync.dma_start(out=x_all, in_=bass.AP(tensor=x_d, offset=0, ap=[[16, 128], [128 * 16, nrb], [1, 16]]))
            dkA = dkpool.tile([128, nrb, 16], dt.float32)
            dkB = dkpool.tile([128, nrb, 16], dt.float32)
            dkC2 = dkpool.tile([128, nrb, 16], dt.float32)
            dkD = dkpool.tile([128, nrb, 16], dt.float32)
            nc.vector.tensor_scalar(out=dkA, in0=x_all, scalar1=4097.0, scalar2=None, op0=Alu.mult)      # c
            nc.vector.tensor_tensor(out=dkB, in0=dkA, in1=x_all, op=Alu.subtract)                        # u = c - x
            nc.vector.tensor_tensor(out=dkA, in0=dkA, in1=dkB, op=Alu.subtract)                          # xh = c - u
            nc.vector.tensor_tensor(out=dkB, in0=x_all, in1=dkA, op=Alu.subtract)                        # xl
            nc.vector.tensor_scalar(out=dkC2, in0=x_all, scalar1=rh, scalar2=None, op0=Alu.mult)         # p
            nc.vector.scalar_tensor_tensor(out=dkD, in0=dkA, scalar=rhh, in1=dkC2, op0=Alu.mult, op1=Alu.subtract)
            nc.vector.scalar_tensor_tensor(out=dkD, in0=dkA, scalar=rhl, in1=dkD, op0=Alu.mult, op1=Alu.add)
            nc.vector.scalar_tensor_tensor(out=dkD, in0=dkB, scalar=rhh, in1=dkD, op0=Alu.mult, op1=Alu.add)
            nc.vector.scalar_tensor_tensor(out=dkD, in0=dkB, scalar=rhl, in1=dkD, op0=Alu.mult, op1=Alu.add)
            nc.vector.scalar_tensor_tensor(out=dkD, in0=x_all, scalar=rl, in1=dkD, op0=Alu.mult, op1=Alu.add)
            nc.vector.tensor_tensor(out=xs_all, in0=dkC2, in1=dkD, op=Alu.add)
            dkpool_cm.__exit__(None, None, None)
            work = ctx.enter_context(tc.tile_pool(name="work", bufs=3))

            for it in range(n_iters):
                row0 = it * 128 * R
                # --- v = xs - c ---
                v_t = work.tile([128, R, 32, 16], dt.float32)
                xs_sl = xs_all[:, it * R:(it + 1) * R, :]
                xs_b = bass.AP(tensor=xs_sl.tensor, offset=xs_sl.offset, ap=[xs_sl.ap[0], [16, R], [0, 32], [1, 16]])
                cb_b = _bcast(cb_t, [[0, R], [16, 32], [1, 16]])
                nc.vector.tensor_tensor(out=v_t, in0=xs_b, in1=cb_b, op=Alu.subtract)

                # t = v + MAGIC (ACT); g = t - MAGIC (ACT)
                t_t = work.tile([128, R, 32, 16], dt.float32)
                nc.scalar.activation(out=t_t, in_=v_t, func=Act.Copy, bias=MAGIC, scale=1.0)
                g_t = work.tile([128, R, 32, 16], dt.float32)
                nc.scalar.activation(out=g_t, in_=t_t, func=Act.Copy, bias=-MAGIC, scale=1.0)

                # eneg = g - v   (exact via Sterbenz; = X - x); overwrites v in place
                e_t = v_t
                nc.vector.tensor_tensor(out=e_t, in0=g_t, in1=v_t, op=Alu.subtract)

                # sq = eneg^2 (ACT); overwrites t in place
                sq_t = t_t
                nc.scalar.activation(out=sq_t, in_=e_t, func=Act.Square, scale=1.0)

                # per-candidate reductions
                D0 = work.tile([128, R, 32], dt.float32)
                nc.vector.tensor_reduce(out=D0, in_=sq_t, axis=AX.X, op=Alu.add)
                P2 = work.tile([128, R, 32], dt.float32)
                nc.vector.tensor_reduce(out=P2, in_=g_t, axis=AX.X, op=Alu.add)
                M = work.tile([128, R, 32], dt.float32)
                nc.vector.tensor_reduce(out=M, in_=e_t, axis=AX.X, op=Alu.max, apply_absolute_value=True)

                # w = g + c (candidate points X); overwrites g in place
                w_t = g_t
                nc.gpsimd.tensor_tensor(out=w_t, in0=g_t, in1=cb_b, op=Alu.add)


                # parity: h = P2/4; odd <=> h is an odd multiple of 0.5
                h_t = work.tile([128, R, 32], dt.float32)
                nc.vector.tensor_scalar(out=h_t, in0=P2, scalar1=0.25, scalar2=None, op0=Alu.mult)
                th_t = work.tile([128, R, 32], dt.float32)
                nc.scalar.activation(out=th_t, in_=h_t, func=Act.Copy, bias=MAGIC1, scale=1.0)
                hr_t = work.tile([128, R, 32], dt.float32)
                nc.scalar.activation(out=hr_t, in_=th_t, func=Act.Copy, bias=-MAGIC1, scale=1.0)
                dp_t = work.tile([128, R, 32], dt.float32)
                nc.vector.tensor_tensor(out=dp_t, in0=h_t, in1=hr_t, op=Alu.subtract)
                o2_t = work.tile([128, R, 32], dt.float32)  # 0.25 if odd else 0
                nc.scalar.activation(out=o2_t, in_=dp_t, func=Act.Square, scale=1.0)

                # Dq = 4*D0 + (64 - 64*M) * o2
                W64 = work.tile([128, R, 32], dt.float32)
                nc.vector.tensor_scalar(out=W64, in0=M, scalar1=-64.0, scalar2=64.0, op0=Alu.mult, op1=Alu.add)
                pen = work.tile([128, R, 32], dt.float32)
                nc.vector.tensor_tensor(out=pen, in0=W64, in1=o2_t, op=Alu.mult)
                Dq = work.tile([128, R, 32], dt.float32)
                nc.vector.scalar_tensor_tensor(out=Dq, in0=D0, scalar=4.0, in1=pen, op0=Alu.mult, op1=Alu.add)

                # first argmin -> one-hot
                Dmin = work.tile([128, R], dt.float32)
                nc.vector.tensor_reduce(out=Dmin, in_=Dq, axis=AX.X, op=Alu.min)
                eq = work.tile([128, R, 32], dt.float32)
                nc.vector.tensor_tensor(out=eq, in0=Dq, in1=_bcast(Dmin, [[1, R], [0, 32]]), op=Alu.is_equal)
                m1 = work.tile([128, R, 32], dt.float32)
                nc.vector.tensor_tensor(out=m1, in0=eq, in1=_bcast(i32_t, [[0, R], [1, 32]]), op=Alu.mult)
                km = work.tile([128, R], dt.float32)
                nc.vector.tensor_reduce(out=km, in_=m1, axis=AX.X, op=Alu.min)
                onehot = work.tile([128, R, 32], dt.float32)
                nc.vector.tensor_tensor(
                    out=onehot, in0=_bcast(i32_t, [[0, R], [1, 32]]), in1=_bcast(km, [[1, R], [0, 32]]), op=Alu.is_equal
                )

                # masked selects (transposed write then grouped reduce over k)
                oh_b = _bcast(onehot, [[32, R], [1, 32], [0, 16]])

                wT = work.tile([128, R, 16, 32], dt.float32)
                wT_w = bass.AP(tensor=wT.tensor, offset=wT.offset, ap=[wT.ap[0], [512, R], [1, 32], [32, 16]])
                nc.gpsimd.tensor_tensor(out=wT_w, in0=w_t, in1=oh_b, op=Alu.mult)
                wsel = work.tile([128, R, 16], dt.float32)
                nc.vector.tensor_reduce(out=wsel, in_=wT, axis=AX.X, op=Alu.add)

                eT = work.tile([128, R, 16, 32], dt.float32)
                eT_w = bass.AP(tensor=eT.tensor, offset=eT.offset, ap=[eT.ap[0], [512, R], [1, 32], [32, 16]])
                nc.gpsimd.tensor_tensor(out=eT_w, in0=e_t, in1=oh_b, op=Alu.mult)
                esel = work.tile([128, R, 16], dt.float32)
                nc.vector.tensor_reduce(out=esel, in_=eT, axis=AX.X, op=Alu.add)

                o2m = work.tile([128, R, 32], dt.float32)
                nc.gpsimd.tensor_tensor(out=o2m, in0=o2_t, in1=onehot, op=Alu.mult)
                o2sel = work.tile([128, R], dt.float32)  # 0.25 if odd else 0
                nc.vector.tensor_reduce(out=o2sel, in_=o2m, axis=AX.X, op=Alu.add)

                # parity flip at first argmax|eneg|
                ae = work.tile([128, R, 16], dt.float32)
                nc.scalar.activation(out=ae, in_=esel, func=Act.Abs, scale=1.0)
                M16 = work.tile([128, R], dt.float32)
                nc.vector.tensor_reduce(out=M16, in_=ae, axis=AX.X, op=Alu.max)
                meq = work.tile([128, R, 16], dt.float32)
                nc.vector.tensor_tensor(out=meq, in0=ae, in1=_bcast(M16, [[1, R], [0, 16]]), op=Alu.is_equal)
                m2 = work.tile([128, R, 16], dt.float32)
                nc.vector.tensor_tensor(out=m2, in0=meq, in1=_bcast(i16_t, [[0, R], [1, 16]]), op=Alu.mult)
                jm = work.tile([128, R], dt.float32)
                nc.vector.tensor_reduce(out=jm, in_=m2, axis=AX.X, op=Alu.min)
                mask1 = work.tile([128, R, 16], dt.float32)
                nc.vector.tensor_tensor(
                    out=mask1, in0=_bcast(i16_t, [[0, R], [1, 16]]), in1=_bcast(jm, [[1, R], [0, 16]]), op=Alu.is_equal
                )
                sgn = work.tile([128, R, 16], dt.float32)
                nc.scalar.activation(out=sgn, in_=esel, func=Act.Sign, scale=1.0)
                u1 = work.tile([128, R, 16], dt.float32)
                nc.vector.tensor_tensor(out=u1, in0=mask1, in1=sgn, op=Alu.mult)
                ohalf = work.tile([128, R], dt.float32)  # -2 if odd else 0
                nc.vector.tensor_scalar(out=ohalf, in0=o2sel, scalar1=-8.0, scalar2=None, op0=Alu.mult)
                u2 = work.tile([128, R, 16], dt.float32)
                nc.vector.tensor_tensor(out=u2, in0=u1, in1=_bcast(ohalf, [[1, R], [0, 16]]), op=Alu.mult)
                Xf = work.tile([128, R, 16], dt.float32)
                nc.vector.tensor_tensor(out=Xf, in0=wsel, in1=u2, op=Alu.add)
                y_t = work.tile([128, R, 16], dt.float32)
                nc.vector.tensor_scalar(out=y_t, in0=Xf, scalar1=float(f32(a_val)), scalar2=None, op0=Alu.mult)

                nc.sync.dma_start(
                    out=bass.AP(tensor=y_d, offset=row0 * 16, ap=[[16, 128], [128 * 16, R], [1, 16]]),
                    in_=y_t,
                )
    nc.finalize()
    return nc


_CACHE = {}


def _get_nc(rows, a_val):
    key = (rows, a_val)
    if key not in _CACHE:
        _CACHE[key] = _build(rows, a_val)
    return _CACHE[key]


def kernel(x_in, C_rep, a):
    from concourse.bass_utils import run_bass_kernel_spmd

    x = np.ascontiguousarray(np.asarray(x_in, dtype=np.float32))
    C = np.asarray(C_rep, dtype=np.float32)
    a_val = float(np.asarray(a).reshape(-1)[0])
    B = x.shape[0]
    rows = B // N_CORES
    assert rows * N_CORES == B

    nc = _get_nc(rows, a_val)

    cb_np = C.reshape(-1).astype(np.float32)
    i32_np = (np.arange(32) - 64).astype(np.float32)
    i16_np = (np.arange(16) - 32).astype(np.float32)
    shards = x.reshape(N_CORES, rows, 16)
    in_maps = [
        {"x": shards[i], "cb": cb_np, "i32": i32_np, "i16": i16_np}
        for i in range(N_CORES)
    ]
    res = run_bass_kernel_spmd(nc, in_maps, core_ids=list(range(N_CORES)))
    y = np.concatenate([res.results[i]["y"] for i in range(N_CORES)], axis=0)
    return y.astype(np.float32)


if __name__ == "__main__":
    rng = np.random.default_rng(0)
    x = rng.standard_normal((262144, 16), dtype=np.float32)
    C = rng.integers(0, 5, size=(32, 16)).astype(np.float32)
    a = np.array([0.59460354], dtype=np.float32)
    y = kernel(x, C, a)
    print("ok", y.shape, y.dtype)


# revision 12
# speedup vs baseline: 1.1463x; 1.0322x over previous
"""Barnes-Wall (BW16) lattice quantizer kernel for Trainium2, 8-core data-parallel.

Algorithm (validated bit-exact vs the jax reference):
  x = x_in / a   (correctly-rounded via Dekker-product division: p = x*rh plus
                  exact product error + x*rl correction, rh+rl ~ 1/a in double)
  For each of 32 codebook rows c: v = x - c, g = 2*round(v/2) (RNE, via the
  +1.5*2^24 magic-number trick which rounds v to the nearest even integer),
  eneg = g - v (= X - x, exact), D0 = sum(eneg^2), P2 = sum(g) (exact),
  M = max|eneg|.  Parity (sum f odd) is derived from P2/4 vs its RNE rounding.
  Parity-odd candidates pay a flip penalty: D ~ 4*D0 + odd*(16-16M).
  Winner k = first argmin; its X/eneg/parity are extracted with a one-hot
  masked reduce; the parity flip is applied at the first argmax|eneg|
  coordinate with direction -sign(eneg); y = X' * a.
"""
import sys

sys.path.insert(0, "/opt/trn_rl_repo")
import contextlib

import numpy as np

import concourse.bass as bass
import concourse.bacc as bacc
import concourse.mybir as mybir
import concourse.tile as tile

f32 = np.float32
MAGIC = float(f32(1.5 * 2.0**24))   # round-to-even-integer magic
MAGIC1 = float(f32(1.5 * 2.0**23))  # round-to-integer magic (parity)

dt = mybir.dt
Alu = mybir.AluOpType
Act = mybir.ActivationFunctionType
AX = mybir.AxisListType

N_CORES = 8
R = 4  # row blocks of 128 per iteration


def _bcast(ap, pattern):
    return bass.AP(tensor=ap.tensor, offset=ap.offset, ap=[ap.ap[0]] + pattern)


def _div_consts(a_val):
    """rh + rl ~ 1/a (double-float), rhh + rhl = Veltkamp split of rh."""
    r64 = 1.0 / np.float64(f32(a_val))
    rh = f32(r64)
    rl = f32(r64 - np.float64(rh))
    c_ = f32(rh * f32(4097.0))
    rhh = f32(c_ - f32(c_ - rh))
    rhl = f32(rh - rhh)
    return float(rh), float(rl), float(rhh), float(rhl)


def _build(rows, a_val):
    nc = bacc.Bacc("TRN2", target_bir_lowering=False)
    x_d = nc.dram_tensor("x", [rows, 16], dt.float32, kind="ExternalInput")
    cb_d = nc.dram_tensor("cb", [512], dt.float32, kind="ExternalInput")
    i32_d = nc.dram_tensor("i32", [32], dt.float32, kind="ExternalInput")
    i16_d = nc.dram_tensor("i16", [16], dt.float32, kind="ExternalInput")
    y_d = nc.dram_tensor("y", [rows, 16], dt.float32, kind="ExternalOutput")

    rh, rl, rhh, rhl = _div_consts(a_val)

    n_iters = rows // (128 * R)
    assert n_iters * 128 * R == rows

    with tile.TileContext(nc) as tc:
        with contextlib.ExitStack() as ctx:
            singles = ctx.enter_context(tc.tile_pool(name="singles", bufs=1))

            cb_t = singles.tile([128, 512], dt.float32)
            nc.sync.dma_start(out=cb_t, in_=bass.AP(tensor=cb_d, offset=0, ap=[[0, 128], [1, 512]]))
            i32_t = singles.tile([128, 32], dt.float32)
            nc.sync.dma_start(out=i32_t, in_=bass.AP(tensor=i32_d, offset=0, ap=[[0, 128], [1, 32]]))
            i16_t = singles.tile([128, 16], dt.float32)
            nc.sync.dma_start(out=i16_t, in_=bass.AP(tensor=i16_d, offset=0, ap=[[0, 128], [1, 16]]))

            # prologue: xs_all = x / a for the whole shard (Dekker, correctly rounded)
            nrb = rows // 128
            xs_all = singles.tile([128, nrb, 16], dt.float32)
            dkpool_cm = tc.tile_pool(name="dk", bufs=1)
            dkpool = dkpool_cm.__enter__()
            x_all = dkpool.tile([128, nrb, 16], dt.float32)
            nc.sync.dma_start(out=x_all, in_=bass.AP(tensor=x_d, offset=0, ap=[[16, 128], [128 * 16, nrb], [1, 16]]))
            dkA = dkpool.tile([128, nrb, 16], dt.float32)
            dkB = dkpool.tile([128, nrb, 16], dt.float32)
            dkC2 = dkpool.tile([128, nrb, 16], dt.float32)
            dkD = dkpool.tile([128, nrb, 16], dt.float32)
            nc.vector.tensor_scalar(out=dkA, in0=x_all, scalar1=4097.0, scalar2=None, op0=Alu.mult)      # c
            nc.vector.tensor_tensor(out=dkB, in0=dkA, in1=x_all, op=Alu.subtract)                        # u = c - x
            nc.vector.tensor_tensor(out=dkA, in0=dkA, in1=dkB, op=Alu.subtract)                          # xh = c - u
            nc.vector.tensor_tensor(out=dkB, in0=x_all, in1=dkA, op=Alu.subtract)                        # xl
            nc.vector.tensor_scalar(out=dkC2, in0=x_all, scalar1=rh, scalar2=None, op0=Alu.mult)         # p
            nc.vector.scalar_tensor_tensor(out=dkD, in0=dkA, scalar=rhh, in1=dkC2, op0=Alu.mult, op1=Alu.subtract)
            nc.vector.scalar_tensor_tensor(out=dkD, in0=dkA, scalar=rhl, in1=dkD, op0=Alu.mult, op1=Alu.add)
            nc.vector.scalar_tensor_tensor(out=dkD, in0=dkB, scalar=rhh, in1=dkD, op0=Alu.mult, op1=Alu.add)
            nc.vector.scalar_tensor_tensor(out=dkD, in0=dkB, scalar=rhl, in1=dkD, op0=Alu.mult, op1=Alu.add)
            nc.vector.scalar_tensor_tensor(out=dkD, in0=x_all, scalar=rl, in1=dkD, op0=Alu.mult, op1=Alu.add)
            nc.vector.tensor_tensor(out=xs_all, in0=dkC2, in1=dkD, op=Alu.add)
            dkpool_cm.__exit__(None, None, None)
            work = ctx.enter_context(tc.tile_pool(name="work", bufs=3))

            for it in range(n_iters):
                row0 = it * 128 * R
                # --- v = xs - c ---
                v_t = work.tile([128, R, 32, 16], dt.float32)
                xs_sl = xs_all[:, it * R:(it + 1) * R, :]
                xs_b = bass.AP(tensor=xs_sl.tensor, offset=xs_sl.offset, ap=[xs_sl.ap[0], [16, R], [0, 32], [1, 16]])
                cb_b = _bcast(cb_t, [[0, R], [16, 32], [1, 16]])
                nc.vector.tensor_tensor(out=v_t, in0=xs_b, in1=cb_b, op=Alu.subtract)

                # t = v + MAGIC (ACT); g = t - MAGIC (ACT)
                t_t = work.tile([128, R, 32, 16], dt.float32)
                nc.scalar.activation(out=t_t, in_=v_t, func=Act.Copy, bias=MAGIC, scale=1.0)
                g_t = work.tile([128, R, 32, 16], dt.float32)
                nc.scalar.activation(out=g_t, in_=t_t, func=Act.Copy, bias=-MAGIC, scale=1.0)

                # eneg = g - v   (exact via Sterbenz; = X - x); overwrites v in place
                e_t = v_t
                nc.vector.tensor_tensor(out=e_t, in0=g_t, in1=v_t, op=Alu.subtract)

                # sq = eneg^2 (ACT); overwrites t in place
                sq_t = t_t
                nc.scalar.activation(out=sq_t, in_=e_t, func=Act.Square, scale=1.0)

                # per-candidate reductions
                D0 = work.tile([128, R, 32], dt.float32)
                nc.vector.tensor_reduce(out=D0, in_=sq_t, axis=AX.X, op=Alu.add)
                P2 = work.tile([128, R, 32], dt.float32)
                nc.vector.tensor_reduce(out=P2, in_=g_t, axis=AX.X, op=Alu.add)
                M = work.tile([128, R, 32], dt.float32)
                nc.vector.tensor_reduce(out=M, in_=e_t, axis=AX.X, op=Alu.max, apply_absolute_value=True)

                # w = g + c (candidate points X); overwrites g in place
                w_t = g_t
                nc.gpsimd.tensor_tensor(out=w_t, in0=g_t, in1=cb_b, op=Alu.add)


                # parity: h = P2/4; odd <=> h is an odd multiple of 0.5
                h_t = work.tile([128, R, 32], dt.float32)
                nc.vector.tensor_scalar(out=h_t, in0=P2, scalar1=0.25, scalar2=None, op0=Alu.mult)
                th_t = work.tile([128, R, 32], dt.float32)
                nc.scalar.activation(out=th_t, in_=h_t, func=Act.Copy, bias=MAGIC1, scale=1.0)
                hr_t = work.tile([128, R, 32], dt.float32)
                nc.scalar.activation(out=hr_t, in_=th_t, func=Act.Copy, bias=-MAGIC1, scale=1.0)
                dp_t = work.tile([128, R, 32], dt.float32)
                nc.vector.tensor_tensor(out=dp_t, in0=h_t, in1=hr_t, op=Alu.subtract)
                o2_t = work.tile([128, R, 32], dt.float32)  # 0.25 if odd else 0
                nc.scalar.activation(out=o2_t, in_=dp_t, func=Act.Square, scale=1.0)

                # Dq = 4*D0 + (64 - 64*M) * o2
                W64 = work.tile([128, R, 32], dt.float32)
                nc.vector.tensor_scalar(out=W64, in0=M, scalar1=-64.0, scalar2=64.0, op0=Alu.mult, op1=Alu.add)
                pen = work.tile([128, R, 32], dt.float32)
                nc.vector.tensor_tensor(out=pen, in0=W64, in1=o2_t, op=Alu.mult)
                Dq = work.tile([128, R, 32], dt.float32)
                nc.vector.scalar_tensor_tensor(out=Dq, in0=D0, scalar=4.0, in1=pen, op0=Alu.mult, op1=Alu.add)

                # first argmin -> one-hot
                Dmin = work.tile([128, R], dt.float32)
                nc.vector.tensor_reduce(out=Dmin, in_=Dq, axis=AX.X, op=Alu.min)
                eq = work.tile([128, R, 32], dt.float32)
                nc.vector.tensor_tensor(out=eq, in0=Dq, in1=_bcast(Dmin, [[1, R], [0, 32]]), op=Alu.is_equal)
                m1 = work.tile([128, R, 32], dt.float32)
                nc.vector.tensor_tensor(out=m1, in0=eq, in1=_bcast(i32_t, [[0, R], [1, 32]]), op=Alu.mult)
                km = work.tile([128, R], dt.float32)
                nc.vector.tensor_reduce(out=km, in_=m1, axis=AX.X, op=Alu.min)
                onehot = work.tile([128, R, 32], dt.float32)
                nc.vector.tensor_tensor(
                    out=onehot, in0=_bcast(i32_t, [[0, R], [1, 32]]), in1=_bcast(km, [[1, R], [0, 32]]), op=Alu.is_equal
                )

                # masked selects (transposed write then grouped reduce over k)
                oh_b = _bcast(onehot, [[32, R], [1, 32], [0, 16]])

                wT = work.tile([128, R, 16, 32], dt.float32)
                wT_w = bass.AP(tensor=wT.tensor, offset=wT.offset, ap=[wT.ap[0], [512, R], [1, 32], [32, 16]])
                nc.gpsimd.tensor_tensor(out=wT_w, in0=w_t, in1=oh_b, op=Alu.mult)
                wsel = work.tile([128, R, 16], dt.float32)
                nc.vector.tensor_reduce(out=wsel, in_=wT, axis=AX.X, op=Alu.add)

                eT = work.tile([128, R, 16, 32], dt.float32)
                eT_w = bass.AP(tensor=eT.tensor, offset=eT.offset, ap=[eT.ap[0], [512, R], [1, 32], [32, 16]])
                nc.gpsimd.tensor_tensor(out=eT_w, in0=e_t, in1=oh_b, op=Alu.mult)
                esel = work.tile([128, R, 16], dt.float32)
                nc.vector.tensor_reduce(out=esel, in_=eT, axis=AX.X, op=Alu.add)

                o2m = work.tile([128, R, 32], dt.float32)
                nc.gpsimd.tensor_tensor(out=o2m, in0=o2_t, in1=onehot, op=Alu.mult)
                o2sel = work.tile([128, R], dt.float32)  # 0.25 if odd else 0
                nc.vector.tensor_reduce(out=o2sel, in_=o2m, axis=AX.X, op=Alu.add)

                # parity flip at first argmax|eneg|
                ae = work.tile([128, R, 16], dt.float32)
                nc.scalar.activation(out=ae, in_=esel, func=Act.Abs, scale=1.0)
                M16 = work.tile([128, R], dt.float32)
                nc.vector.tensor_reduce(out=M16, in_=ae, axis=AX.X, op=Alu.max)
                meq = work.tile([128, R, 16], dt.float32)
                nc.vector.tensor_tensor(out=meq, in0=ae, in1=_bcast(M16, [[1, R], [0, 16]]), op=Alu.is_equal)
                m2 = work.tile([128, R, 16], dt.float32)
                nc.vector.tensor_tensor(out=m2, in0=meq, in1=_bcast(i16_t, [[0, R], [1, 16]]), op=Alu.mult)
                jm = work.tile([128, R], dt.float32)
                nc.vector.tensor_reduce(out=jm, in_=m2, axis=AX.X, op=Alu.min)
                mask1 = work.tile([128, R, 16], dt.float32)
                nc.vector.tensor_tensor(
                    out=mask1, in0=_bcast(i16_t, [[0, R], [1, 16]]), in1=_bcast(jm, [[1, R], [0, 16]]), op=Alu.is_equal
                )
                sgn = work.tile([128, R, 16], dt.float32)
                nc.scalar.activation(out=sgn, in_=esel, func=Act.Sign, scale=1.0)
                u1 = work.tile([128, R, 16], dt.float32)
                nc.vector.tensor_tensor(out=u1, in0=mask1, in1=sgn, op=Alu.mult)
                ohalf = work.tile([128, R], dt.float32)  # -2 if odd else 0
                nc.vector.tensor_scalar(out=ohalf, in0=o2sel, scalar1=-8.0, scalar2=None, op0=Alu.mult)
                u2 = work.tile([128, R, 16], dt.float32)
                nc.vector.tensor_tensor(out=u2, in0=u1, in1=_bcast(ohalf, [[1, R], [0, 16]]), op=Alu.mult)
                Xf = work.tile([128, R, 16], dt.float32)
                nc.vector.tensor_tensor(out=Xf, in0=wsel, in1=u2, op=Alu.add)
                y_t = work.tile([128, R, 16], dt.float32)
                nc.vector.tensor_scalar(out=y_t, in0=Xf, scalar1=float(f32(a_val)), scalar2=None, op0=Alu.mult)

                nc.sync.dma_start(
                    out=bass.AP(tensor=y_d, offset=row0 * 16, ap=[[16, 128], [128 * 16, R], [1, 16]]),
                    in_=y_t,
                )
    nc.finalize()
    return nc


_CACHE = {}


def _get_nc(rows, a_val):
    key = (rows, a_val)
    if key not in _CACHE:
        _CACHE[key] = _build(rows, a_val)
    return _CACHE[key]


def kernel(x_in, C_rep, a):
    from concourse.bass_utils import run_bass_kernel_spmd

    x = np.ascontiguousarray(np.asarray(x_in, dtype=np.float32))
    C = np.asarray(C_rep, dtype=np.float32)
    a_val = float(np.asarray(a).reshape(-1)[0])
    B = x.shape[0]
    rows = B // N_CORES
    assert rows * N_CORES == B

    nc = _get_nc(rows, a_val)

    cb_np = C.reshape(-1).astype(np.float32)
    i32_np = (np.arange(32) - 64).astype(np.float32)
    i16_np = (np.arange(16) - 32).astype(np.float32)
    shards = x.reshape(N_CORES, rows, 16)
    in_maps = [
        {"x": shards[i], "cb": cb_np, "i32": i32_np, "i16": i16_np}
        for i in range(N_CORES)
    ]
    res = run_bass_kernel_spmd(nc, in_maps, core_ids=list(range(N_CORES)))
    y = np.concatenate([res.results[i]["y"] for i in range(N_CORES)], axis=0)
    return y.astype(np.float32)


if __name__ == "__main__":
    rng = np.random.default_rng(0)
    x = rng.standard_normal((262144, 16), dtype=np.float32)
    C = rng.integers(0, 5, size=(32, 16)).astype(np.float32)
    a = np.array([0.59460354], dtype=np.float32)
    y = kernel(x, C, a)
    print("ok", y.shape, y.dtype)
